# revision 1
# baseline (speedup 1.0000x reference)
"""Bass/Trainium2 kernel for nn_BBoxDetectionLoss (YOLO-style bbox detection loss).

Strategy (pure data parallel over 8 NeuronCores, 4 images per core):
  The loss decomposes as
    noobj = 0.5 * (sum_all softplus(obj_pred) - sum_resp softplus(obj_pred)) / n_neg
    obj   =        sum_resp softplus(-obj_pred) / n_pos
    coord = 5 *    sum_resp |bbox_pred - target|^2 / n_pos
  where "resp" is at most 24 cells per image (one per gt box, deduped last-wins).
  So the only dense work is a softplus-sum over the obj channel; the responsible
  cells are fetched with one indirect (gather) DMA per core and everything else
  is tiny per-box arithmetic on 4 partitions.  Per-core partial sums are packed
  into an 8-float vector, AllReduce'd across the 8 cores, and each core applies
  the final normalization.
"""

import math
import os
import sys

import numpy as np

for _p in ("/opt/trn_rl_repo",):
    if _p not in sys.path:
        sys.path.insert(0, _p)

import concourse.bass as bass
import concourse.tile as tile
from concourse import bacc, mybir
from concourse.bass_utils import run_bass_kernel_spmd
from concourse.bass import _add_dep_helper

F32 = mybir.dt.float32
I32 = mybir.dt.int32

N_CORES = 8
B, H, W, A, C = 32, 112, 112, 9, 5
NBOX = 24
BL = B // N_CORES                     # images per core = 4
CELLS_L = BL * H * W * A              # 451584 cells per core
ELEMS_L = CELLS_L * C                 # 2257920 f32 per core
P = 128
FPL = ELEMS_L // P                    # 17640 elements per partition
CELLS_PP = CELLS_L // P               # 3528 cells per partition
NCHUNK = 4
CH_CELL = CELLS_PP // NCHUNK          # 441 cells per partition per chunk
CH_EL = CH_CELL * C                   # 2205 elems per partition per chunk
TOT_CELLS = B * H * W * A             # 3612672 (for n_neg)

LAMBDA_COORD = 5.0
LAMBDA_NOOBJ = 0.5

# debug bisection flags
USE_AR = os.environ.get("K_USE_AR", "1") == "1"
USE_GATHER = os.environ.get("K_USE_GATHER", "1") == "1"
USE_BOX = os.environ.get("K_USE_BOX", "1") == "1"
USE_DIV = os.environ.get("K_USE_DIV", "1") == "1"

# ---- host-side constants ---------------------------------------------------


def _anchors():
    a = []
    for s in (32, 64, 128):
        for r in (0.5, 1.0, 2.0):
            a.append(
                (
                    np.float32(s * math.sqrt(r) / 224.0),
                    np.float32(s / math.sqrt(r) / 224.0),
                )
            )
    return np.array(a, np.float32)  # [9, 2]


# const tensor layout, [4, KCONST] f32:
#   [0:216)     AWB    anchor w, broadcast per (box i, anchor a), a inner
#   [216:432)   AHB    anchor h
#   [432:648)   AWAHB  aw*ah (f32 product, bit-identical to reference's)
#   [648:864)   IOTA9B float(a)
#   [864:1080)  RAWB   1/aw  (f32)
#   [1080:1296) RAHB   1/ah  (f32)
#   [1296:1872) JGT    pair mask [i, j] (i-major, 24x24): 1.0 iff j > i
#   [1872:1873) BASE   per-partition cell base = p * H*W*A
KCONST = 1880


def _build_const():
    anc = _anchors()
    aw, ah = anc[:, 0], anc[:, 1]
    awah = (aw * ah).astype(np.float32)
    raw = (np.float32(1.0) / aw).astype(np.float32)
    rah = (np.float32(1.0) / ah).astype(np.float32)
    row = np.zeros(KCONST, np.float32)
    row[0:216] = np.tile(aw, NBOX)
    row[216:432] = np.tile(ah, NBOX)
    row[432:648] = np.tile(awah, NBOX)
    row[648:864] = np.tile(np.arange(9, dtype=np.float32), NBOX)
    row[864:1080] = np.tile(raw, NBOX)
    row[1080:1296] = np.tile(rah, NBOX)
    jgt = (np.arange(NBOX)[None, :] > np.arange(NBOX)[:, None]).astype(np.float32)
    row[1296:1872] = jgt.reshape(-1)
    cst = np.broadcast_to(row, (BL, KCONST)).copy()
    cst[:, 1872] = np.arange(BL, dtype=np.float32) * (H * W * A)
    return cst


# ---- bass program ----------------------------------------------------------

MAGIC = 8388608.0  # 2^23: (x + 2^23) - 2^23 rounds x to nearest integer
SPLIT = 4097.0     # 2^12 + 1: Dekker split constant for f32

_DIV_UID = [0]


def _dtile(sm, shape):
    _DIV_UID[0] += 1
    return sm.tile(shape, F32, name=f"dv{_DIV_UID[0]}", tag=f"dv{_DIV_UID[0]}")


def _two_prod_err(nc, sm, q, qh, ql, bh, bl, b_ap, shape):
    """err = q*b - fl(q*b) exactly (Dekker); returns (p, err) tiles."""
    p = _dtile(sm, shape)
    nc.vector.tensor_tensor(out=p[:], in0=q[:], in1=b_ap, op=mybir.AluOpType.mult)
    e = _dtile(sm, shape)
    t = _dtile(sm, shape)
    nc.vector.tensor_mul(e[:], qh[:], bh[:])
    nc.vector.tensor_sub(e[:], e[:], p[:])
    nc.vector.tensor_mul(t[:], qh[:], bl[:])
    nc.vector.tensor_add(e[:], e[:], t[:])
    nc.vector.tensor_mul(t[:], ql[:], bh[:])
    nc.vector.tensor_add(e[:], e[:], t[:])
    nc.vector.tensor_mul(t[:], ql[:], bl[:])
    nc.vector.tensor_add(e[:], e[:], t[:])
    return p, e


def _dekker_split(nc, sm, x_ap, shape):
    """x = xh + xl with xh having <=12 mantissa bits; exact products follow."""
    c = _dtile(sm, shape)
    nc.vector.tensor_scalar_mul(c[:], x_ap, SPLIT)
    u = _dtile(sm, shape)
    nc.vector.tensor_tensor(out=u[:], in0=c[:], in1=x_ap, op=mybir.AluOpType.subtract)
    xh = _dtile(sm, shape)
    nc.vector.tensor_sub(xh[:], c[:], u[:])
    xl = _dtile(sm, shape)
    nc.vector.tensor_tensor(out=xl[:], in0=x_ap, in1=xh[:], op=mybir.AluOpType.subtract)
    return xh, xl


def _exact_div(nc, sm, a_ap, b_ap, shape, dbg=None):
    """q = RN(a/b) bit-exact (positive a, normal b), matching IEEE f32 divide.

    DVE reciprocal is correctly rounded (verified on HW), so q0 = fl(a*RN(1/b))
    is within ~1 ulp of a/b.  The residual r = a - q0*b is computed exactly via
    Dekker TwoProd (no FMA needed); the Newton correction c = r*rec then has
    ~1e-7-ulp error, and the final f32 add q = fl(q0 + c) performs the correct
    rounding itself.  Verified bit-exact vs numpy f32 divide on 10M samples.
    """
    rec = _dtile(sm, shape)
    nc.vector.reciprocal(rec[:], b_ap)
    q0 = _dtile(sm, shape)
    nc.vector.tensor_tensor(out=q0[:], in0=a_ap, in1=rec[:], op=mybir.AluOpType.mult)

    bh, bl = _dekker_split(nc, sm, b_ap, shape)
    qh, ql = _dekker_split(nc, sm, q0[:], shape)
    p, e = _two_prod_err(nc, sm, q0, qh, ql, bh, bl, b_ap, shape)
    r = _dtile(sm, shape)
    nc.vector.tensor_tensor(out=r[:], in0=a_ap, in1=p[:], op=mybir.AluOpType.subtract)
    nc.vector.tensor_sub(r[:], r[:], e[:])
    nc.vector.tensor_mul(r[:], r[:], rec[:])
    q = _dtile(sm, shape)
    nc.vector.tensor_add(q[:], q0[:], r[:])
    return q


# Force exp and ln onto the single combined ACT table set: strip them from
# every other set (indices preserved; act_func_set_id is positional) so
# Bacc's table-load pass emits one ACT_TABLE_LOAD instead of ping-ponging
# between exp_and_others and natural_log on every chunk (~1.3us per load).
def _patch_act_tables():
    import functools

    import concourse.bacc as _bacc
    import concourse.hw_specs as _hs

    orig = _hs.get_activation_tables

    @functools.cache
    def patched(arch):
        t = {k: set(v) for k, v in orig(arch).items()}
        keep = "natural_log_exp_and_others"
        strip = {mybir.ActivationFunctionType.Exp, mybir.ActivationFunctionType.Ln}
        if keep in t and strip <= t[keep]:
            for k in t:
                if k != keep:
                    t[k] = t[k] - strip
        return t

    _bacc.get_activation_tables = patched


_patch_act_tables()


def _build_nc():
    nc = bacc.Bacc(
        "TRN2", target_bir_lowering=False, debug=False, num_devices=N_CORES
    )

    pred = nc.dram_tensor("pred", [ELEMS_L], F32, kind="ExternalInput")
    bbt = nc.dram_tensor("bb", [BL, NBOX * 4], F32, kind="ExternalInput")
    cstt = nc.dram_tensor("cst", [BL, KCONST], F32, kind="ExternalInput")
    outt = nc.dram_tensor("out", [5], F32, kind="ExternalOutput")
    partsd = nc.dram_tensor("parts", [1, 8], F32, kind="ExternalOutput")
    gd = nc.dram_tensor("gdump", [BL, NBOX * C], F32, kind="ExternalOutput")
    offd = nc.dram_tensor("offdump", [BL, NBOX], I32, kind="ExternalOutput")

    predv = pred[:].rearrange("(p f) -> p f", p=P)          # [128, 17640]
    gatherv = pred[:].rearrange("(n c) -> n c", c=C)        # [451584, 5]

    with tile.TileContext(nc) as tc:
        with (
            tc.tile_pool(name="big", bufs=3) as big,
            tc.tile_pool(name="small", bufs=1) as sm,
            tc.tile_pool(name="psum", bufs=1, space="PSUM") as pp,
            tc.tile_pool(name="dram", bufs=1, space="DRAM") as dp,
        ):
            if USE_BOX:
                # ---------------- stage A: box targets (4 partitions) ----------
                bb = sm.tile([BL, NBOX * 4], F32)
                nc.sync.dma_start(out=bb[:], in_=bbt[:])
                cst = sm.tile([BL, KCONST], F32)
                nc.sync.dma_start(out=cst[:], in_=cstt[:])

                bb3 = bb[:].rearrange("p (i c) -> p i c", c=4)
                cxv, cyv, wv, hv = (bb3[:, :, k] for k in range(4))
                AWB = cst[:, 0:216]
                AHB = cst[:, 216:432]
                AWAHB = cst[:, 432:648]
                IOTA9B = cst[:, 648:864]
                RAWB = cst[:, 864:1080]
                RAHB = cst[:, 1080:1296]
                JGT = cst[:, 1296:1872]
                BASE = cst[:, 1872:1873]

                sx = sm.tile([BL, NBOX], F32)
                sy = sm.tile([BL, NBOX], F32)
                nc.vector.tensor_scalar_mul(sx[:], cxv, float(W))
                nc.vector.tensor_scalar_mul(sy[:], cyv, float(H))
                # floor via 2^23 round-trip (RN) + correction, then clip to [0, W-1]
                gx = sm.tile([BL, NBOX], F32)
                gy = sm.tile([BL, NBOX], F32)
                corr = sm.tile([BL, NBOX], F32)
                for gv, sv, hi in ((gx, sx, W - 1), (gy, sy, H - 1)):
                    nc.vector.tensor_scalar(
                        gv[:], sv[:], MAGIC, -MAGIC,
                        op0=mybir.AluOpType.add, op1=mybir.AluOpType.add,
                    )
                    nc.vector.tensor_tensor(
                        out=corr[:], in0=gv[:], in1=sv[:], op=mybir.AluOpType.is_gt
                    )
                    nc.vector.tensor_sub(gv[:], gv[:], corr[:])
                    nc.vector.tensor_scalar(
                        gv[:], gv[:], float(hi), 0.0,
                        op0=mybir.AluOpType.min, op1=mybir.AluOpType.max,
                    )
                tx = sm.tile([BL, NBOX], F32)
                ty = sm.tile([BL, NBOX], F32)
                nc.vector.tensor_sub(tx[:], sx[:], gx[:])
                nc.vector.tensor_sub(ty[:], sy[:], gy[:])

                # validity: any coord nonzero
                vmax = sm.tile([BL, NBOX], F32)
                nc.vector.tensor_reduce(
                    vmax[:], bb3, axis=mybir.AxisListType.X,
                    op=mybir.AluOpType.max, apply_absolute_value=True,
                )
                valid = sm.tile([BL, NBOX], F32)
                nc.vector.tensor_scalar(
                    valid[:], vmax[:], 0.0, None, op0=mybir.AluOpType.is_gt
                )

                # IoU against 9 anchors -> best (first max wins).  The quotient must
                # be bit-exact IEEE f32 division: exact ties between anchors decide
                # argmax, and the reference breaks them by first-index.
                t216a = sm.tile([BL, 216], F32)
                t216b = sm.tile([BL, 216], F32)
                w9 = wv.to_broadcast([BL, NBOX, 9])
                h9 = hv.to_broadcast([BL, NBOX, 9])
                a3 = lambda ap: ap.rearrange("p (i a) -> p i a", a=9)
                nc.vector.tensor_tensor(
                    out=a3(t216a[:]), in0=w9, in1=a3(AWB), op=mybir.AluOpType.min
                )
                nc.vector.tensor_tensor(
                    out=a3(t216b[:]), in0=h9, in1=a3(AHB), op=mybir.AluOpType.min
                )
                nc.vector.tensor_mul(t216a[:], t216a[:], t216b[:])  # inter
                wh = sm.tile([BL, NBOX], F32)
                nc.vector.tensor_mul(wh[:], wv, hv)
                nc.vector.tensor_tensor(
                    out=a3(t216b[:]), in0=wh[:].to_broadcast([BL, NBOX, 9]),
                    in1=a3(AWAHB), op=mybir.AluOpType.add,
                )
                nc.vector.tensor_sub(t216b[:], t216b[:], t216a[:])  # union
                nc.vector.tensor_scalar_add(t216b[:], t216b[:], 1e-16)
                if USE_DIV:
                    iou = _exact_div(nc, sm, t216a[:], t216b[:], [BL, 216])
                else:
                    iou = sm.tile([BL, 216], F32, name="iou_t", tag="iou_t")
                    nc.vector.reciprocal(iou[:], t216b[:])
                    nc.vector.tensor_mul(iou[:], iou[:], t216a[:])

                ioumax = sm.tile([BL, NBOX], F32)
                nc.vector.tensor_reduce(
                    ioumax[:], a3(iou[:]), axis=mybir.AxisListType.X,
                    op=mybir.AluOpType.max,
                )
                # val = eq ? a : 9  ->  val = eq * (a - 9) + 9 ; best = min(val)
                nc.vector.tensor_tensor(
                    out=a3(t216a[:]), in0=a3(iou[:]),
                    in1=ioumax[:].to_broadcast([BL, NBOX, 9]),
                    op=mybir.AluOpType.is_equal,
                )
                nc.vector.tensor_scalar_add(t216b[:], IOTA9B, -9.0)
                nc.vector.tensor_mul(t216b[:], t216a[:], t216b[:])
                nc.vector.tensor_scalar_add(t216b[:], t216b[:], 9.0)
                best = sm.tile([BL, NBOX], F32)
                nc.vector.tensor_reduce(
                    best[:], a3(t216b[:]), axis=mybir.AxisListType.X,
                    op=mybir.AluOpType.min,
                )

                # one-hot select of 1/aw, 1/ah
                nc.vector.tensor_tensor(
                    out=a3(t216a[:]), in0=a3(IOTA9B),
                    in1=best[:].to_broadcast([BL, NBOX, 9]),
                    op=mybir.AluOpType.is_equal,
                )
                rawsel = sm.tile([BL, NBOX], F32)
                rahsel = sm.tile([BL, NBOX], F32)
                nc.vector.tensor_mul(t216b[:], t216a[:], RAWB)
                nc.vector.tensor_reduce(
                    rawsel[:], a3(t216b[:]), axis=mybir.AxisListType.X,
                    op=mybir.AluOpType.add,
                )
                nc.vector.tensor_mul(t216b[:], t216a[:], RAHB)
                nc.vector.tensor_reduce(
                    rahsel[:], a3(t216b[:]), axis=mybir.AxisListType.X,
                    op=mybir.AluOpType.add,
                )
                # tw = ln(w/aw + 1e-16), th = ln(h/ah + 1e-16)
                twv = sm.tile([BL, NBOX], F32)
                thv = sm.tile([BL, NBOX], F32)
                nc.vector.tensor_mul(twv[:], wv, rawsel[:])
                nc.vector.tensor_mul(thv[:], hv, rahsel[:])
                nc.vector.tensor_scalar_add(twv[:], twv[:], 1e-16)
                nc.vector.tensor_scalar_add(thv[:], thv[:], 1e-16)
                nc.scalar.activation(twv[:], twv[:], mybir.ActivationFunctionType.Ln)
                nc.scalar.activation(thv[:], thv[:], mybir.ActivationFunctionType.Ln)

                # cell id and flat offsets
                cellf = sm.tile([BL, NBOX], F32)
                nc.vector.tensor_scalar_mul(cellf[:], gy[:], float(W))
                nc.vector.tensor_add(cellf[:], cellf[:], gx[:])
                nc.vector.tensor_scalar_mul(cellf[:], cellf[:], float(A))
                nc.vector.tensor_add(cellf[:], cellf[:], best[:])
                offf = sm.tile([BL, NBOX], F32)
                nc.vector.tensor_scalar(
                    offf[:], cellf[:], BASE, None, op0=mybir.AluOpType.add
                )
                offi = sm.tile([BL, NBOX], I32)
                nc.vector.tensor_copy(offi[:], offf[:])

                # dedup: box i dies if a later valid box j hits the same cell
                p3 = lambda ap: ap.rearrange("p (i j) -> p i j", j=NBOX)
                eqp = sm.tile([BL, NBOX * NBOX], F32)
                nc.vector.tensor_tensor(
                    out=p3(eqp[:]),
                    in0=cellf[:].to_broadcast([BL, NBOX, NBOX]),
                    in1=cellf[:][:, None, :].broadcast_to([BL, NBOX, NBOX]),
                    op=mybir.AluOpType.is_equal,
                )
                nc.vector.tensor_mul(eqp[:], eqp[:], JGT)
                nc.vector.tensor_tensor(
                    out=p3(eqp[:]), in0=p3(eqp[:]),
                    in1=valid[:][:, None, :].broadcast_to([BL, NBOX, NBOX]),
                    op=mybir.AluOpType.mult,
                )
                dead = sm.tile([BL, NBOX], F32)
                nc.vector.tensor_reduce(
                    dead[:], p3(eqp[:]), axis=mybir.AxisListType.X,
                    op=mybir.AluOpType.max,
                )
                live = sm.tile([BL, NBOX], F32)
                nc.vector.tensor_mul(live[:], valid[:], dead[:])
                nc.vector.tensor_sub(live[:], valid[:], live[:])

                npos_p = sm.tile([BL, 1], F32)
                nc.vector.tensor_reduce(
                    npos_p[:], live[:], axis=mybir.AxisListType.X,
                    op=mybir.AluOpType.add,
                )

                # gather responsible predictions: one indirect DMA, 96 rows of 5
                g = sm.tile([BL, NBOX * C], F32)
                if USE_GATHER:
                    # HW indirect DMA consumes ONE offset per partition row, so
                    # spread the 96 boxes across 96 partitions for the gather.
                    off96 = sm.tile([BL * NBOX, 1], I32)
                    nc.sync.dma_start(out=off96[:], in_=offi[:])
                    g96 = sm.tile([BL * NBOX, C], F32)
                    nc.gpsimd.indirect_dma_start(
                        out=g96[:],
                        out_offset=None,
                        in_=gatherv,
                        in_offset=bass.IndirectOffsetOnAxis(ap=off96[:], axis=0),
                    )
                    nc.sync.dma_start(out=g[:], in_=g96[:])
                else:
                    nc.gpsimd.memset(g[:], 0.0)
                nc.sync.dma_start(out=gd[:], in_=g[:])
                nc.sync.dma_start(out=offd[:], in_=offi[:])
                g5 = g[:].rearrange("p (i c) -> p i c", c=C)

                # gathered-cell softplus: Exp now, Ln after the dense Exps
                spn = sm.tile([BL, NBOX], F32)
                spp = sm.tile([BL, NBOX], F32)
                nc.scalar.activation(
                    spn[:], g5[:, :, 4], mybir.ActivationFunctionType.Exp, scale=-1.0
                )
                nc.scalar.activation(
                    spn[:], spn[:], mybir.ActivationFunctionType.Ln, bias=1.0
                )
                nc.scalar.activation(
                    spp[:], g5[:, :, 4], mybir.ActivationFunctionType.Exp
                )
                nc.scalar.activation(
                    spp[:], spp[:], mybir.ActivationFunctionType.Ln, bias=1.0
                )
                obj_p = sm.tile([BL, 1], F32)
                sub_p = sm.tile([BL, 1], F32)
                spl = sm.tile([BL, NBOX], F32)
                nc.vector.tensor_mul(spl[:], spn[:], live[:])
                nc.vector.tensor_reduce(
                    obj_p[:], spl[:], axis=mybir.AxisListType.X,
                    op=mybir.AluOpType.add,
                )
                spl2 = sm.tile([BL, NBOX], F32)
                nc.vector.tensor_mul(spl2[:], spp[:], live[:])
                nc.vector.tensor_reduce(
                    sub_p[:], spl2[:], axis=mybir.AxisListType.X,
                    op=mybir.AluOpType.add,
                )

                # coord = sum_c (pred_c - t_c)^2, masked by live
                d = sm.tile([BL, NBOX * 4], F32)
                d3 = d[:].rearrange("p (i c) -> p i c", c=4)
                for cidx, tv in enumerate((tx, ty, twv, thv)):
                    nc.vector.tensor_tensor(
                        out=d3[:, :, cidx], in0=g5[:, :, cidx], in1=tv[:],
                        op=mybir.AluOpType.subtract,
                    )
                nc.vector.tensor_mul(d[:], d[:], d[:])
                cb = sm.tile([BL, NBOX], F32)
                nc.vector.tensor_reduce(
                    cb[:], d3, axis=mybir.AxisListType.X, op=mybir.AluOpType.add
                )
                coord_p = sm.tile([BL, 1], F32)
                cbl = sm.tile([BL, NBOX], F32)
                nc.vector.tensor_mul(cbl[:], cb[:], live[:])
                nc.vector.tensor_reduce(
                    coord_p[:], cbl[:], axis=mybir.AxisListType.X,
                    op=mybir.AluOpType.add,
                )

            else:
                coord_p = sm.tile([BL, 1], F32, name="coord_p0", tag="coord_p0")
                obj_p = sm.tile([BL, 1], F32, name="obj_p0", tag="obj_p0")
                sub_p = sm.tile([BL, 1], F32, name="sub_p0", tag="sub_p0")
                npos_p = sm.tile([BL, 1], F32, name="npos_p0", tag="npos_p0")
                for _t in (coord_p, obj_p, sub_p, npos_p):
                    nc.gpsimd.memset(_t[:], 0.0)

            # ---------------- stage B: dense softplus over obj channel -----
            # softplus(x) = ln(exp(x) + 1); exp and ln share one ACT table set
            accs = sm.tile([P, NCHUNK], F32)
            for i in range(NCHUNK):
                chunk = big.tile([P, CH_EL], F32, tag="chunk")
                nc.sync.dma_start(
                    out=chunk[:], in_=predv[:, i * CH_EL : (i + 1) * CH_EL]
                )
                sp = big.tile([P, CH_CELL], F32, tag="sp")
                ch4 = chunk[:, 4::5]
                nc.scalar.activation(
                    sp[:], ch4, mybir.ActivationFunctionType.Exp
                )
                nc.scalar.activation(
                    sp[:], sp[:], mybir.ActivationFunctionType.Ln, bias=1.0,
                    accum_out=accs[:, i : i + 1],
                )
            dense_col = sm.tile([P, 1], F32)
            nc.vector.tensor_reduce(
                dense_col[:], accs[:], axis=mybir.AxisListType.X,
                op=mybir.AluOpType.add,
            )

            # ---------------- stage C: pack partials, matmul-reduce, AR ----
            rhs = sm.tile([P, 8], F32)
            nc.gpsimd.memset(rhs[:], 0.0)
            nc.vector.tensor_copy(rhs[:, 0:1], dense_col[:])
            nc.vector.tensor_copy(rhs[0:BL, 1:2], sub_p[:])
            nc.vector.tensor_copy(rhs[0:BL, 2:3], obj_p[:])
            nc.vector.tensor_copy(rhs[0:BL, 3:4], coord_p[:])
            nc.vector.tensor_copy(rhs[0:BL, 4:5], npos_p[:])
            ones = sm.tile([P, 1], F32)
            nc.gpsimd.memset(ones[:], 1.0)
            ps = pp.tile([1, 8], F32)
            nc.tensor.matmul(ps[:], lhsT=ones[:], rhs=rhs[:], start=True, stop=True)
            ar_sb = sm.tile([1, 8], F32)
            nc.vector.tensor_copy(ar_sb[:], ps[:])
            nc.sync.dma_start(out=partsd[:], in_=ar_sb[:])

            arr = sm.tile([1, 8], F32)
            if USE_AR:
                ar_in = dp.tile([1, 8], F32)
                ar_out = dp.tile([1, 8], F32)
                nc.sync.dma_start(out=ar_in[:], in_=ar_sb[:])
                nc.gpsimd.collective_compute(
                    "AllReduce",
                    mybir.AluOpType.add,
                    replica_groups=[list(range(N_CORES))],
                    ins=[ar_in[:].opt()],
                    outs=[ar_out[:].opt()],
                )
                nc.sync.dma_start(out=arr[:], in_=ar_out[:])
            else:
                nc.vector.tensor_copy(arr[:], ar_sb[:])

            # ---------------- stage D: final normalization ------------------
            # arr = [dense, sub, obj, coord, npos, _, _, _] (global sums)
            den = sm.tile([1, 2], F32)
            # den[0] = max(npos, 1); den[1] = max(TOT_CELLS - npos, 1)
            nc.vector.tensor_scalar(
                den[:, 0:1], arr[:, 4:5], 1.0, None, op0=mybir.AluOpType.max
            )
            nc.vector.tensor_scalar(
                den[:, 1:2], arr[:, 4:5], -1.0, float(TOT_CELLS),
                op0=mybir.AluOpType.mult, op1=mybir.AluOpType.add,
            )
            nc.vector.tensor_scalar(
                den[:, 1:2], den[:, 1:2], 1.0, None, op0=mybir.AluOpType.max
            )
            rec = sm.tile([1, 2], F32)
            nc.vector.reciprocal(rec[:], den[:])

            res = sm.tile([1, 8], F32)
            nc.gpsimd.memset(res[:], 0.0)
            # coord
            nc.vector.tensor_tensor(
                out=res[:, 1:2], in0=arr[:, 3:4], in1=rec[:, 0:1],
                op=mybir.AluOpType.mult,
            )
            nc.vector.tensor_scalar_mul(res[:, 1:2], res[:, 1:2], LAMBDA_COORD)
            # obj
            nc.vector.tensor_tensor(
                out=res[:, 2:3], in0=arr[:, 2:3], in1=rec[:, 0:1],
                op=mybir.AluOpType.mult,
            )
            # noobj = 0.5 * (dense - sub) / n_neg
            nc.vector.tensor_sub(res[:, 3:4], arr[:, 0:1], arr[:, 1:2])
            nc.vector.tensor_tensor(
                out=res[:, 3:4], in0=res[:, 3:4], in1=rec[:, 1:2],
                op=mybir.AluOpType.mult,
            )
            nc.vector.tensor_scalar_mul(res[:, 3:4], res[:, 3:4], LAMBDA_NOOBJ)
            # total
            nc.vector.tensor_add(res[:, 0:1], res[:, 1:2], res[:, 2:3])
            nc.vector.tensor_add(res[:, 0:1], res[:, 0:1], res[:, 3:4])

            nc.sync.dma_start(out=outt[:], in_=res[0:1, 0:5])

    nc.compile()
    return nc


_NC_CACHE = None


def _get_nc():
    global _NC_CACHE
    if _NC_CACHE is None:
        _NC_CACHE = _build_nc()
    return _NC_CACHE


def kernel_with_results(predictions, bboxes, **run_kwargs):
    predictions = np.ascontiguousarray(predictions, dtype=np.float32)
    bboxes = np.ascontiguousarray(bboxes, dtype=np.float32)
    assert predictions.shape == (B, H, W, A, C)
    assert bboxes.shape == (B, NBOX, 4)

    cst = _build_const()
    in_maps = []
    for c in range(N_CORES):
        shard_p = predictions[c * BL : (c + 1) * BL].reshape(-1)
        shard_b = bboxes[c * BL : (c + 1) * BL].reshape(BL, NBOX * 4)
        in_maps.append({"pred": shard_p, "bb": shard_b, "cst": cst})

    nc = _get_nc()
    res = run_bass_kernel_spmd(nc, in_maps, core_ids=list(range(N_CORES)), **run_kwargs)
    out = np.asarray(res.results[0]["out"], dtype=np.float32).reshape(5)
    return out, res


def kernel(predictions, bboxes):
    out, _ = kernel_with_results(predictions, bboxes)
    return out

